# revision 1
# baseline (speedup 1.0000x reference)
"""GatedAttentionPooling Trainium2 kernel.

z[b] = sum_{i in bag b} softmax_bag(alpha)_i * x_i
alpha_i = (tanh(x W1^T) * softmax_h(x W2^T)) @ W3^T

Strategy: data-parallel over 8 cores (even row split; sorted batch ids).
Per core, per 128-row tile:
  - two GEMMs (fp16 operands, fp32 PSUM) -> u|v logits
  - ACT tanh / exp(+fused row-sum), DVE ops -> alpha -> e = exp(alpha)
    (no max subtraction needed: |alpha| <= max|W3| ~ 0.044)
  - pooling matmul: (onehot * e)^T @ x accumulated in PSUM over all tiles
Host merges per-core partial sums and exp-sums linearly (exact).
"""

import numpy as np
import ml_dtypes

BF16 = ml_dtypes.bfloat16
N = 262144
D = 1024
H = 512
B = 512
NCORES = 8
ROWS = N // NCORES          # 32768 rows per core
P = 128                     # partitions / tile rows
MAXB = 128                  # max local bags per core (padded)

_CACHE = {}
TRACE = False
LAST_RESULT = None


def _build_program(n_tiles):
    import concourse.bass as bass
    import concourse.bacc as bacc
    import concourse.mybir as mybir
    import concourse.tile as tile

    dt = mybir.dt
    AF = mybir.ActivationFunctionType
    ALU = mybir.AluOpType

    nc = bacc.Bacc("TRN2", target_bir_lowering=False, debug=False,
                   num_devices=NCORES)

    rows = n_tiles * P
    xp = nc.dram_tensor("xp", (rows, D), dt.bfloat16, kind="ExternalInput")
    # pre-transposed x: xpt[t, d, c*128+i] = x[t*128+i, c*128+d]
    xpt = nc.dram_tensor("xpt", (n_tiles, P, D), dt.bfloat16,
                         kind="ExternalInput")
    oneh = nc.dram_tensor("oneh", (n_tiles, P, MAXB), dt.bfloat16,
                          kind="ExternalInput")
    w12s = nc.dram_tensor("w12s", (P, (D // P) * 2 * H), dt.bfloat16,
                          kind="ExternalInput")
    w3r = nc.dram_tensor("w3r", (P, H), dt.float16, kind="ExternalInput")
    S = nc.dram_tensor("S", (MAXB, D), dt.float32, kind="ExternalOutput")
    E = nc.dram_tensor("E", (P, n_tiles), dt.bfloat16, kind="ExternalOutput")

    KC = D // P  # 8 contraction chunks

    with tile.TileContext(nc) as tc:
        with (
            tc.tile_pool(name="const", bufs=1) as constp,
            tc.tile_pool(name="xt", bufs=4) as xtp,
            tc.tile_pool(name="xn", bufs=4) as xnp_,
            tc.tile_pool(name="oh", bufs=4) as ohp,
            tc.tile_pool(name="work", bufs=4) as workp,
            tc.tile_pool(name="uvps", bufs=3, space=bass.MemorySpace.PSUM) as psp,
            tc.tile_pool(name="accps", bufs=1, space=bass.MemorySpace.PSUM) as psaccp,
        ):
            w12 = constp.tile([P, KC * 2 * H], dt.bfloat16)
            nc.gpsimd.dma_start(w12[:], w12s.ap())
            w3 = constp.tile([P, H], dt.float16)
            nc.gpsimd.dma_start(w3[:], w3r.ap())
            ebuf = constp.tile([P, n_tiles], dt.bfloat16)
            pool_acc = psaccp.tile([MAXB, D], dt.float32)

            for t in range(n_tiles):
                # transposed x tile: xt[d, c*128+i] = x[t*128+i, c*128+d]
                xt = xtp.tile([P, D], dt.bfloat16)
                nc.gpsimd.dma_start(xt[:], xpt[t])
                # natural x tile (pooling rhs)
                xn = xnp_.tile([P, D], dt.bfloat16)
                nc.gpsimd.dma_start(xn[:], xp[t * P:(t + 1) * P, :])
                oh = ohp.tile([P, MAXB], dt.bfloat16)
                nc.gpsimd.dma_start(oh[:], oneh[t])

                uv = psp.tile([P, 2 * H], dt.float32)
                for c in range(KC):
                    lhs = xt[:, c * P:(c + 1) * P]
                    nc.tensor.matmul(uv[:, 0:H], lhs,
                                     w12[:, c * 2 * H:c * 2 * H + H],
                                     start=(c == 0), stop=(c == KC - 1))
                    nc.tensor.matmul(uv[:, H:2 * H], lhs,
                                     w12[:, c * 2 * H + H:(c + 1) * 2 * H],
                                     start=(c == 0), stop=(c == KC - 1))

                u16 = workp.tile([P, H], dt.float16)
                nc.scalar.activation(u16[:], uv[:, 0:H], AF.Tanh)
                ev = workp.tile([P, H], dt.float16)
                den = workp.tile([P, 1], dt.float32)
                nc.scalar.activation(ev[:], uv[:, H:2 * H], AF.Exp,
                                     accum_out=den[:])
                uw = workp.tile([P, H], dt.float16)
                nc.vector.tensor_tensor(uw[:], u16[:], w3[:], ALU.mult)
                prod = workp.tile([P, H], dt.float16)
                nc.vector.tensor_tensor(prod[:], uw[:], ev[:], ALU.mult)
                num = workp.tile([P, 1], dt.float32)
                nc.vector.reduce_sum(num[:], prod[:], mybir.AxisListType.X)
                rden = workp.tile([P, 1], dt.float32)
                nc.vector.reciprocal(rden[:], den[:])
                # e = exp(num * rden)
                ef = workp.tile([P, 1], dt.float32)
                nc.scalar.activation(ef[:], num[:], AF.Exp, scale=rden[:])
                nc.scalar.copy(ebuf[:, t:t + 1], ef[:])
                lhsp = workp.tile([P, MAXB], dt.bfloat16)
                nc.vector.tensor_scalar_mul(lhsp[:], oh[:], ef[:])
                nc.tensor.matmul(pool_acc[:, 0:H], lhsp[:], xn[:, 0:H],
                                 start=(t == 0), stop=(t == n_tiles - 1),
                                 skip_group_check=True)
                nc.tensor.matmul(pool_acc[:, H:D], lhsp[:], xn[:, H:D],
                                 start=(t == 0), stop=(t == n_tiles - 1),
                                 skip_group_check=True)

            sout = constp.tile([MAXB, D], dt.float32)
            nc.scalar.copy(sout[:], pool_acc[:])
            nc.gpsimd.dma_start(S.ap(), sout[:])
            nc.gpsimd.dma_start(E.ap(), ebuf[:])

    nc.compile()
    return nc


def _get_program(n_tiles):
    if n_tiles not in _CACHE:
        _CACHE[n_tiles] = _build_program(n_tiles)
    return _CACHE[n_tiles]


def kernel(x, batch, W1, W2, W3):
    global LAST_RESULT
    from concourse import bass_utils

    x = np.asarray(x)
    batch = np.asarray(batch)
    W1 = np.asarray(W1, dtype=np.float32)
    W2 = np.asarray(W2, dtype=np.float32)
    W3 = np.asarray(W3, dtype=np.float32)

    # shared weight layouts
    w12t = np.concatenate([W1.T, W2.T], axis=1)              # (D, 2H)
    w12s = np.ascontiguousarray(
        w12t.reshape(D // P, P, 2 * H).transpose(1, 0, 2).reshape(P, -1)
    ).astype(BF16)
    w3r = np.ascontiguousarray(
        np.broadcast_to(W3.reshape(1, H), (P, H))).astype(np.float16)

    x16 = x.astype(BF16)

    n_tiles = ROWS // P
    in_maps = []
    bases = []
    locals_ = []
    for c in range(NCORES):
        ids = batch[c * ROWS:(c + 1) * ROWS].astype(np.int64)
        base = int(ids[0])
        local = (ids - base).astype(np.int64)
        nb = int(local.max()) + 1
        assert nb <= MAXB, f"core {c}: {nb} local bags > {MAXB}"
        oneh = np.zeros((ROWS, MAXB), dtype=BF16)
        oneh[np.arange(ROWS), local] = BF16(1.0)
        xs = x16[c * ROWS:(c + 1) * ROWS]
        xpt = np.ascontiguousarray(
            xs.reshape(n_tiles, P, D // P, P).transpose(0, 3, 2, 1)
            .reshape(n_tiles, P, D))
        in_maps.append({
            "xp": np.ascontiguousarray(xs),
            "xpt": xpt,
            "oneh": np.ascontiguousarray(oneh.reshape(n_tiles, P, MAXB)),
            "w12s": w12s,
            "w3r": w3r,
        })
        bases.append(base)
        locals_.append(local)

    nc = _get_program(n_tiles)
    res = bass_utils.run_bass_kernel_spmd(
        nc, in_maps, core_ids=list(range(NCORES)), trace=TRACE)
    LAST_RESULT = res

    Z = np.zeros((B, D), dtype=np.float64)
    DEN = np.zeros((B,), dtype=np.float64)
    for c in range(NCORES):
        Sc = np.asarray(res.results[c]["S"], dtype=np.float64)
        Ec = np.asarray(res.results[c]["E"], dtype=np.float64)
        e_flat = Ec.T.reshape(-1)                             # row order
        local = locals_[c]
        nb = int(local.max()) + 1
        den = np.bincount(local, weights=e_flat, minlength=nb)[:nb]
        Z[bases[c]:bases[c] + nb] += Sc[:nb]
        DEN[bases[c]:bases[c] + nb] += den
    out = np.zeros((B, D), dtype=np.float32)
    nzero = DEN > 0
    out[nzero] = (Z[nzero] / DEN[nzero, None]).astype(np.float32)
    return out



# revision 8
# speedup vs baseline: 1.1393x; 1.1393x over previous
"""GatedAttentionPooling Trainium2 kernel (fp8 DoubleRow edition).

z[b] = sum_{i in bag b} softmax_bag(alpha)_i * x_i
alpha_i = (tanh(x W1^T) * softmax_h(x W2^T)) @ W3^T

Strategy: data-parallel over 8 cores (even row split; sorted batch ids).
Per core, per 128-row tile:
  - two GEMMs in fp8e4m3 with DoubleRow perf mode (2 k-subtiles per
    matmul, 2x PE throughput); weights pre-scaled by 4096 on host,
    un-scaled inside the activations.
  - ACT tanh / exp(+fused row-sum), DVE ops -> alpha -> e = exp(alpha)
    (no max subtraction needed: |alpha| <= max|W3| ~ 0.044)
  - pooling matmul (bf16): (onehot * e)^T @ x accumulated in PSUM over
    all tiles; software-pipelined one tile behind the main GEMMs.
All per-tile input data (fp8 x-transposed | bf16 x | bf16 onehot) is
packed into a single 3328 B/partition DMA issued from the Sync queue.
Host merges per-core partial sums and exp-sums linearly (exact).
"""

import numpy as np
import ml_dtypes

BF16 = ml_dtypes.bfloat16
FP8 = ml_dtypes.float8_e4m3
N = 262144
D = 1024
H = 512
B = 512
NCORES = 8
ROWS = N // NCORES          # 32768 rows per core
P = 128                     # partitions / tile rows
MAXB = 128                  # max local bags per core (padded)
KC2 = D // 256              # 4 double-row contraction chunks
WSCALE = 4096.0
INV_WSCALE = 1.0 / WSCALE

# packed per-tile byte layout: fp8 xT | bf16 x | bf16 onehot
XQ_B = D                    # 1024 bytes
XN_B = 2 * D                # 2048 bytes
OH_B = 2 * MAXB             # 256 bytes
PK_B = XQ_B + XN_B + OH_B   # 3328 bytes

_CACHE = {}
TRACE = False
LAST_RESULT = None


def _build_program(n_tiles):
    import concourse.bass as bass
    import concourse.bacc as bacc
    import concourse.mybir as mybir
    import concourse.tile as tile

    dt = mybir.dt
    AF = mybir.ActivationFunctionType
    ALU = mybir.AluOpType
    DR = mybir.MatmulPerfMode.DoubleRow

    nc = bacc.Bacc("TRN2", target_bir_lowering=False, debug=False,
                   num_devices=NCORES)

    pk = nc.dram_tensor("pk", (n_tiles, P, PK_B), dt.uint8,
                        kind="ExternalInput")
    w8s = nc.dram_tensor("w8s", (P, KC2 * 2 * 2 * H), dt.float8e4,
                         kind="ExternalInput")
    w3r = nc.dram_tensor("w3r", (P, H), dt.float16, kind="ExternalInput")
    S = nc.dram_tensor("S", (MAXB, D), dt.float32, kind="ExternalOutput")
    E = nc.dram_tensor("E", (P, n_tiles), dt.float32, kind="ExternalOutput")

    with tile.TileContext(nc) as tc:
        with (
            tc.tile_pool(name="const", bufs=1) as constp,
            tc.tile_pool(name="pkt", bufs=4) as pkp,
            tc.tile_pool(name="work", bufs=4) as workp,
            tc.tile_pool(name="lhs", bufs=4) as lhspp,
            tc.tile_pool(name="uvps", bufs=3, space=bass.MemorySpace.PSUM) as psp,
            tc.tile_pool(name="accps", bufs=1, space=bass.MemorySpace.PSUM) as psaccp,
        ):
            w8 = constp.tile([P, KC2 * 2 * 2 * H], dt.float8e4)
            nc.gpsimd.dma_start(w8[:], w8s.ap())
            w3 = constp.tile([P, H], dt.float16)
            nc.gpsimd.dma_start(w3[:], w3r.ap())
            ebuf = constp.tile([P, n_tiles], dt.float32)
            pool_acc = psaccp.tile([MAXB, D], dt.float32)

            pending = None  # (lhsp, xn_view) for software-pipelined pooling

            for t in range(n_tiles):
                pkt = pkp.tile([P, PK_B], dt.uint8)
                nc.sync.dma_start(pkt[:], pk[t])
                xq = pkt[:, 0:XQ_B].bitcast(dt.float8e4)           # (128,1024)
                xn = pkt[:, XQ_B:XQ_B + XN_B].bitcast(dt.bfloat16)  # (128,1024)
                oh = pkt[:, XQ_B + XN_B:PK_B].bitcast(dt.bfloat16)  # (128,128)

                uv = psp.tile([P, 2 * H], dt.float32)
                for c in range(KC2):
                    lhsT = xq[:, c * 256:(c + 1) * 256].rearrange(
                        "p (i r) -> p i r", i=2)
                    # w8 layout [p, c, half, i, j]: both rhs slices contiguous
                    rhs_u = w8[:, c * 4 * H:c * 4 * H + 2 * H].rearrange(
                        "p (i j) -> p i j", i=2)
                    rhs_v = w8[:, c * 4 * H + 2 * H:(c + 1) * 4 * H].rearrange(
                        "p (i j) -> p i j", i=2)
                    nc.tensor.matmul(uv[:, 0:H], lhsT, rhs_u,
                                     start=(c == 0), stop=(c == KC2 - 1),
                                     perf_mode=DR)
                    nc.tensor.matmul(uv[:, H:2 * H], lhsT, rhs_v,
                                     start=(c == 0), stop=(c == KC2 - 1),
                                     perf_mode=DR)

                # pool matmuls for the previous tile (one-tile PE skew)
                if pending is not None:
                    plhs, pxn, pt = pending
                    nc.tensor.matmul(pool_acc[:, 0:H], plhs, pxn[:, 0:H],
                                     start=(pt == 0), stop=False,
                                     skip_group_check=True)
                    nc.tensor.matmul(pool_acc[:, H:D], plhs, pxn[:, H:D],
                                     start=(pt == 0), stop=False,
                                     skip_group_check=True)

                u16 = workp.tile([P, H], dt.float16)
                nc.scalar.activation(u16[:], uv[:, 0:H], AF.Tanh,
                                     scale=INV_WSCALE)
                ev = workp.tile([P, H], dt.float16)
                den = workp.tile([P, 1], dt.float32)
                nc.scalar.activation(ev[:], uv[:, H:2 * H], AF.Exp,
                                     scale=INV_WSCALE, accum_out=den[:])
                uw = workp.tile([P, H], dt.float16)
                nc.vector.tensor_tensor(uw[:], u16[:], w3[:], ALU.mult)
                prod = workp.tile([P, H], dt.float16)
                nc.vector.tensor_tensor(prod[:], uw[:], ev[:], ALU.mult)
                num = workp.tile([P, 1], dt.float32)
                nc.vector.reduce_sum(num[:], prod[:], mybir.AxisListType.X)
                rden = workp.tile([P, 1], dt.float32)
                nc.vector.reciprocal(rden[:], den[:])
                # e = exp(num * rden), written straight into the E row
                nc.scalar.activation(ebuf[:, t:t + 1], num[:], AF.Exp,
                                     scale=rden[:])
                lhsp = lhspp.tile([P, MAXB], dt.bfloat16)
                nc.scalar.mul(lhsp[:], oh[:], ebuf[:, t:t + 1])
                pending = (lhsp, xn, t)

            plhs, pxn, pt = pending
            nc.tensor.matmul(pool_acc[:, 0:H], plhs, pxn[:, 0:H],
                             start=False, stop=True, skip_group_check=True)
            nc.tensor.matmul(pool_acc[:, H:D], plhs, pxn[:, H:D],
                             start=False, stop=True, skip_group_check=True)

            sout = constp.tile([MAXB, D], dt.float32)
            nc.scalar.copy(sout[:], pool_acc[:])
            nc.gpsimd.dma_start(S.ap(), sout[:])
            nc.gpsimd.dma_start(E.ap(), ebuf[:])

    nc.compile()
    return nc


def _get_program(n_tiles):
    if n_tiles not in _CACHE:
        _CACHE[n_tiles] = _build_program(n_tiles)
    return _CACHE[n_tiles]


def kernel(x, batch, W1, W2, W3):
    global LAST_RESULT
    from concourse import bass_utils

    x = np.asarray(x)
    batch = np.asarray(batch)
    W1 = np.asarray(W1, dtype=np.float32)
    W2 = np.asarray(W2, dtype=np.float32)
    W3 = np.asarray(W3, dtype=np.float32)

    n_tiles = ROWS // P

    # shared weight layouts
    wcat = np.concatenate([W1.T, W2.T], axis=1)              # (D, 2H)
    w8 = (wcat * WSCALE).astype(FP8)
    # w8s[p, c, half, i, j] = WSCALE * wcat[256c + 128i + p, 512*half + j]
    w8s = np.ascontiguousarray(
        w8.reshape(KC2, 2, P, 2, H).transpose(2, 0, 3, 1, 4).reshape(P, -1))
    w3r = np.ascontiguousarray(
        np.broadcast_to(W3.reshape(1, H), (P, H))).astype(np.float16)

    x8 = x.astype(FP8)
    x16 = x.astype(BF16)

    in_maps = []
    bases = []
    locals_ = []
    for c in range(NCORES):
        ids = batch[c * ROWS:(c + 1) * ROWS].astype(np.int64)
        base = int(ids[0])
        local = (ids - base).astype(np.int64)
        nb = int(local.max()) + 1
        assert nb <= MAXB, f"core {c}: {nb} local bags > {MAXB}"
        oneh = np.zeros((ROWS, MAXB), dtype=BF16)
        oneh[np.arange(ROWS), local] = BF16(1.0)

        # fp8 transposed x: xq[t, p, 256c+128i+r] = x[t*128+r, 256c+128i+p]
        xq = (x8[c * ROWS:(c + 1) * ROWS]
              .reshape(n_tiles, P, KC2, 2, P).transpose(0, 4, 2, 3, 1))
        xq = np.ascontiguousarray(xq).reshape(n_tiles, P, XQ_B).view(np.uint8)
        xn = np.ascontiguousarray(
            x16[c * ROWS:(c + 1) * ROWS].reshape(n_tiles, P, D))
        xn = xn.view(np.uint8).reshape(n_tiles, P, XN_B)
        oh = oneh.reshape(n_tiles, P, MAXB).view(np.uint8).reshape(
            n_tiles, P, OH_B)
        pk = np.concatenate([xq, xn, oh], axis=2)

        in_maps.append({"pk": pk, "w8s": w8s, "w3r": w3r})
        bases.append(base)
        locals_.append(local)

    nc = _get_program(n_tiles)
    res = bass_utils.run_bass_kernel_spmd(
        nc, in_maps, core_ids=list(range(NCORES)), trace=TRACE)
    LAST_RESULT = res

    Z = np.zeros((B, D), dtype=np.float64)
    DEN = np.zeros((B,), dtype=np.float64)
    for c in range(NCORES):
        Sc = np.asarray(res.results[c]["S"], dtype=np.float64)
        Ec = np.asarray(res.results[c]["E"], dtype=np.float64)
        e_flat = Ec.T.reshape(-1)                             # row order
        local = locals_[c]
        nb = int(local.max()) + 1
        den = np.bincount(local, weights=e_flat, minlength=nb)[:nb]
        Z[bases[c]:bases[c] + nb] += Sc[:nb]
        DEN[bases[c]:bases[c] + nb] += den
    out = np.zeros((B, D), dtype=np.float32)
    nzero = DEN > 0
    out[nzero] = (Z[nzero] / DEN[nzero, None]).astype(np.float32)
    return out
